# revision 11
# baseline (speedup 1.0000x reference)
"""RGCN (2-layer, per-(dst,rel) mean aggregation) + triplet projection,
distributed over 8 Trainium2 NeuronCores (one SPMD Bass/Tile program).

Sharding: nodes are assigned to (core, window, lane) by a host-side
best-fit bin packing so that EVERY (window, rel) run has <= 128 edges
(K=1): chunk (w, r) holds exactly the edges with dst in window w and
type r. W windows/core, C = 8*W chunks per layer pass (~96% slot fill).

Aggregate-first: y[w,r] = msg^T @ slab accumulated in PSUM per window
(slab[e, lane(dst)] = 1/cnt), then agg = x @ root + sum_r y_r @ W_r + b,
ReLU. Layer-1 messages are HOST-pregathered; layer-2 gathers h1[p(src)]
per chunk via indirect DMA (int32 idx, one 128-row gather per chunk).

All large DRAM tensors use the device tile layout directly (partition-
major [128, C, ...]) so every stream DMA is a contiguous multi-KB run
per partition — no scatter/transpose patterns on the DMA queues. The
x^T / h1^T tiles needed for the root-term matmuls are produced with
tensor-engine transposes (identity matmul), not transpose-DMAs.

The u/v projections (u = h2 @ Wp[:256] + bp, v = h2 @ Wp[256:]) are
fused into the layer-2 window loop; h2^T stays in SBUF. Only u is
AllGathered. Triplet per chunk: out = u[p(src)] (indirect gather, same
idx tensor as layer 2) + slabT_chunk^T-matmul of the LOCAL window's v
rows (binary one-hot — no gather and no v AllGather). Output in slot
order; host scatters back to edge order.
"""

import numpy as np
import ml_dtypes

BF16 = ml_dtypes.bfloat16

N, R, F, E, NCORES = 50000, 8, 256, 400000, 8
LAST_EXEC_NS = None


def _plan(src, dst, et, norm):
    """Node->core/window/lane packing + edge slot assignment."""
    deg = np.zeros((N, R), dtype=np.int32)
    np.add.at(deg, (dst, et), 1)
    tot = deg.sum(1)

    # 1) node -> core: greedy balance total edges, node cap N/NCORES
    order = np.argsort(-tot, kind="stable")
    core_of = np.full(N, -1, np.int8)
    core_edges = np.zeros(NCORES, np.int64)
    core_nodes_cnt = np.zeros(NCORES, np.int64)
    cap = N // NCORES
    for n in order:
        cands = np.where(core_nodes_cnt < cap)[0]
        c = cands[np.argmin(core_edges[cands])]
        core_of[n] = c
        core_edges[c] += tot[n]
        core_nodes_cnt[c] += 1

    # 2) per-core window packing (best fit decreasing, fixed W, retry W+1)
    def pack(nodes, W):
        dv = deg[nodes]
        o = np.argsort(-dv.max(1).astype(np.int64) * 100000 - dv.sum(1),
                       kind="stable")
        nodes = nodes[o]
        dv = dv[o]
        ws = np.zeros((W, R), np.int32)
        wc = np.zeros(W, np.int32)
        assign = np.zeros(len(nodes), np.int32)
        for i in range(len(nodes)):
            cand = ws + dv[i]
            feas = (cand <= 128).all(1) & (wc < 128)
            if not feas.any():
                return None
            score = (cand.astype(np.int64) ** 2).sum(1)
            score[~feas] = 1 << 60
            w = int(np.argmin(score))
            assign[i] = w
            ws[w] += dv[i]
            wc[w] += 1
        return nodes, assign

    percore_nodes = [np.where(core_of == c)[0] for c in range(NCORES)]
    lb = 0
    for c in range(NCORES):
        nd = percore_nodes[c]
        lb = max(lb, -(-len(nd) // 128), int(-(-deg[nd].sum(0).max() // 128)))
    W = lb
    packed = None
    while packed is None:
        res = [pack(percore_nodes[c], W) for c in range(NCORES)]
        if all(r is not None for r in res):
            packed = res
        else:
            W += 1

    win_of = np.zeros(N, np.int32)
    lane_of = np.zeros(N, np.int32)
    for c in range(NCORES):
        nodes, assign = packed[c]
        o = np.argsort(assign, kind="stable")
        nodes, assign = nodes[o], assign[o]
        lane = np.arange(len(nodes)) - np.searchsorted(assign, assign)
        win_of[nodes] = assign
        lane_of[nodes] = lane

    # 3) edge -> slot (per dst core, chunk (w, r), position in run)
    C = W * R
    ecore = core_of[dst].astype(np.int64)
    ch = win_of[dst].astype(np.int64) * R + et
    gkey = ecore * C + ch
    eorder = np.argsort(gkey, kind="stable")
    gk = gkey[eorder]
    starts = np.zeros(len(gk), np.int64)
    b = np.flatnonzero(np.diff(gk)) + 1
    off = np.arange(len(gk))
    starts[b] = off[b]
    starts = np.maximum.accumulate(starts)
    pos = off - starts
    assert pos.max() < 128, f"run overflow {pos.max()}"
    slot = (gk % C) * 128 + pos

    slots = []
    gcore = gk // C
    for c in range(NCORES):
        m = gcore == c
        slots.append((eorder[m], slot[m]))
    return W, win_of, lane_of, core_of, slots


def _wrap_idx(idx):
    """int32 [C*128] -> [128, C]: chunk c's 128 rows on partitions."""
    return np.ascontiguousarray(idx.reshape(-1, 128).T)


def _build(W):
    import concourse.bass as bass
    import concourse.bacc as bacc
    import concourse.mybir as mybir
    import concourse.tile as tile

    dt = mybir.dt
    C = W * R
    WP = W * 128           # padded nodes per core
    S = C * 128            # slots per core
    nc = bacc.Bacc("TRN2", target_bir_lowering=False, debug=False,
                   num_devices=NCORES)
    AF = mybir.ActivationFunctionType

    xshT = nc.dram_tensor("xshT", [128, W, 2, 128], dt.bfloat16, kind="ExternalInput")
    msg1 = nc.dram_tensor("msg1", [128, C, F], dt.bfloat16, kind="ExternalInput")
    slab_d = nc.dram_tensor("slab", [128, C, 128], dt.bfloat16, kind="ExternalInput")
    slabT_d = nc.dram_tensor("slabT", [128, C, 128], dt.bfloat16, kind="ExternalInput")
    idx_d = nc.dram_tensor("idx", [128, C], dt.int32, kind="ExternalInput")
    iden_d = nc.dram_tensor("iden", [128, 128], dt.bfloat16, kind="ExternalInput")
    w1d = nc.dram_tensor("w1", [R, F, F], dt.bfloat16, kind="ExternalInput")
    w2d = nc.dram_tensor("w2", [R, F, F], dt.bfloat16, kind="ExternalInput")
    r1d = nc.dram_tensor("r1", [F, F], dt.bfloat16, kind="ExternalInput")
    r2d = nc.dram_tensor("r2", [F, F], dt.bfloat16, kind="ExternalInput")
    b1d = nc.dram_tensor("b1", [128, F], dt.float32, kind="ExternalInput")
    b2d = nc.dram_tensor("b2", [128, F], dt.float32, kind="ExternalInput")
    wpud = nc.dram_tensor("wpu", [F, F], dt.bfloat16, kind="ExternalInput")
    wpvd = nc.dram_tensor("wpv", [F, F], dt.bfloat16, kind="ExternalInput")
    bpd = nc.dram_tensor("bp", [128, F], dt.float32, kind="ExternalInput")
    tout = nc.dram_tensor("tout", [128, C, F], dt.bfloat16, kind="ExternalOutput")

    rg = [list(range(NCORES))]

    with tile.TileContext(nc) as tc:
        with (
            tc.tile_pool(name="const", bufs=1) as cp,
            tc.tile_pool(name="msg", bufs=5) as msgp,
            tc.tile_pool(name="stt", bufs=3) as stp,
            tc.tile_pool(name="yw", bufs=2) as yp,
            tc.tile_pool(name="small", bufs=4) as sp,
            tc.tile_pool(name="out", bufs=3) as op,
            tc.tile_pool(name="ps", bufs=1, space="PSUM") as psp,
            tc.tile_pool(name="psagg", bufs=1, space="PSUM") as psaggp,
            tc.tile_pool(name="dram", bufs=1, space="DRAM") as dram,
        ):
            w_sb = [cp.tile([128, 16, F], dt.bfloat16, tag=f"w{i}", name=f"w{i}")
                    for i in range(2)]
            nc.sync.dma_start(w_sb[0][:], w1d.ap().rearrange("r (h p) o -> p (r h) o", p=128))
            nc.sync.dma_start(w_sb[1][:], w2d.ap().rearrange("r (h p) o -> p (r h) o", p=128))
            rt_sb = [cp.tile([128, 2, F], dt.bfloat16, tag=f"rt{i}", name=f"rt{i}")
                     for i in range(2)]
            nc.sync.dma_start(rt_sb[0][:], r1d.ap().rearrange("(h p) o -> p h o", p=128))
            nc.sync.dma_start(rt_sb[1][:], r2d.ap().rearrange("(h p) o -> p h o", p=128))
            b_sb = [cp.tile([128, F], dt.float32, tag=f"b{i}", name=f"b{i}")
                    for i in range(2)]
            nc.sync.dma_start(b_sb[0][:], b1d[:])
            nc.sync.dma_start(b_sb[1][:], b2d[:])
            wpu_sb = cp.tile([128, 2, F], dt.bfloat16, tag="wpu", name="wpu")
            wpv_sb = cp.tile([128, 2, F], dt.bfloat16, tag="wpv", name="wpv")
            nc.sync.dma_start(wpu_sb[:], wpud.ap().rearrange("(h p) o -> p h o", p=128))
            nc.sync.dma_start(wpv_sb[:], wpvd.ap().rearrange("(h p) o -> p h o", p=128))
            bp_sb = cp.tile([128, F], dt.float32, tag="bp", name="bp")
            nc.sync.dma_start(bp_sb[:], bpd[:])
            idx_sb = cp.tile([128, C], dt.int32, tag="idx", name="idx")
            nc.sync.dma_start(idx_sb[:], idx_d[:])
            iden = cp.tile([128, 128], dt.bfloat16, tag="iden", name="iden")
            nc.sync.dma_start(iden[:], iden_d[:])
            # norm slab cache: used by both layer passes
            slab_sb = cp.tile([128, C, 128], dt.bfloat16, tag="slab", name="slab")
            nc.sync.dma_start(slab_sb[:], slab_d[:])

            h1b = dram.tile([WP, F], dt.bfloat16, tag="h1b", name="h1b")
            h1bT = dram.tile([128, W, 2, 128], dt.bfloat16, tag="h1bT", name="h1bT")
            uloc = dram.tile([WP, F], dt.bfloat16, tag="uloc", name="uloc")
            vloc = dram.tile([128, W, F], dt.bfloat16, tag="vloc", name="vloc")
            h1f = dram.tile([NCORES * WP, F], dt.bfloat16, addr_space="Shared",
                            tag="h1f", name="h1f")
            uf = dram.tile([NCORES * WP, F], dt.bfloat16, addr_space="Shared",
                           tag="uf", name="uf")

            def layer(li, table, hout):
                for w in range(W):
                    mt = msgp.tile([128, R, F], dt.bfloat16, tag="mt", name="mt")
                    if li == 0:
                        eng = nc.sync if w % 2 == 0 else nc.scalar
                        eng.dma_start(mt[:], msg1[:, w * R:(w + 1) * R, :])
                    else:
                        for b in range(R):
                            ch = w * R + b
                            nc.gpsimd.indirect_dma_start(
                                out=mt[:, b, :], out_offset=None, in_=table,
                                in_offset=bass.IndirectOffsetOnAxis(
                                    ap=idx_sb[:, ch:ch + 1], axis=0))
                    xt = sp.tile([128, 2, 128], dt.bfloat16, tag="xt", name="xt")
                    if li == 0:
                        nc.sync.dma_start(xt[:], xshT[:, w, :, :])
                    else:
                        nc.sync.dma_start(xt[:], h1bT[:, w, :, :])
                    # rel halves: matmuls of half B overlap PSUM copies of A
                    yq = [[yp.tile([128, 512], dt.bfloat16, tag=f"yq{fh}{hf}",
                                   name=f"yq{fh}{hf}") for hf in range(2)]
                          for fh in range(2)]
                    for half in range(2):
                        ps = [psp.tile([128, 512], dt.float32, tag=f"ps{fh}{half}",
                                       name=f"ps{fh}{half}") for fh in range(2)]
                        for r4 in range(4):
                            r = half * 4 + r4
                            ch = w * R + r
                            for fh in range(2):
                                nc.tensor.matmul(
                                    ps[fh][:, r4 * 128:r4 * 128 + 128],
                                    lhsT=mt[:, r, fh * 128:(fh + 1) * 128],
                                    rhs=slab_sb[:, ch, :],
                                    start=True, stop=True)
                        nc.vector.tensor_copy(yq[0][half][:], ps[0][:])
                        nc.scalar.copy(yq[1][half][:], ps[1][:])
                    # aggregation: roots first (xt ready early, no yq dep)
                    agg = psaggp.tile([128, F], dt.float32, tag="agg", name="agg")
                    for fh in range(2):
                        nc.tensor.matmul(agg[:], lhsT=xt[:, fh, :],
                                         rhs=rt_sb[li][:, fh, :],
                                         start=(fh == 0), stop=False)
                    for half in range(2):
                        for r4 in range(4):
                            r = half * 4 + r4
                            for fh in range(2):
                                nc.tensor.matmul(
                                    agg[:],
                                    lhsT=yq[fh][half][:, r4 * 128:r4 * 128 + 128],
                                    rhs=w_sb[li][:, r * 2 + fh, :],
                                    start=False,
                                    stop=(half == 1 and r4 == 3 and fh == 1))
                    hfb = sp.tile([128, F], dt.bfloat16, tag="hfb", name="hfb")
                    nc.vector.tensor_tensor(hfb[:], agg[:], b_sb[li][:],
                                            op=mybir.AluOpType.add)
                    hw = sp.tile([128, F], dt.bfloat16, tag="hw", name="hw")
                    nc.scalar.activation(hw[:], hfb[:], AF.Relu)
                    # h^T via tensor-engine transpose of the pre-relu sum;
                    # relu commutes with transpose and is re-applied below
                    pst = psaggp.tile([128, F], dt.bfloat16, tag="pst", name="pst")
                    for fh in range(2):
                        nc.tensor.transpose(pst[:, fh * 128:(fh + 1) * 128],
                                            hfb[:, fh * 128:(fh + 1) * 128],
                                            iden[:])
                    hT = sp.tile([128, 2, 128], dt.bfloat16, tag="hT", name="hT")
                    nc.vector.tensor_scalar_max(
                        hT[:].rearrange("p a b -> p (a b)"), pst[:], 0.0)
                    if li == 0:
                        nc.sync.dma_start(hout[w * 128:(w + 1) * 128, :], hw[:])
                        nc.sync.dma_start(h1bT[:, w, :, :], hT[:])
                    else:
                        # fused u/v projections for this window (h2^T in SBUF)
                        psuv = psaggp.tile([128, 2 * F], dt.float32, tag="psuv",
                                           name="psuv")
                        psu = psuv[:, 0:F]
                        psv = psuv[:, F:2 * F]
                        # chains must not interleave within one PSUM bank
                        for fh in range(2):
                            nc.tensor.matmul(psu, lhsT=hT[:, fh, :],
                                             rhs=wpu_sb[:, fh, :],
                                             start=(fh == 0), stop=(fh == 1))
                        for fh in range(2):
                            nc.tensor.matmul(psv, lhsT=hT[:, fh, :],
                                             rhs=wpv_sb[:, fh, :],
                                             start=(fh == 0), stop=(fh == 1))
                        uo = sp.tile([128, F], dt.bfloat16, tag="uo", name="uo")
                        nc.vector.tensor_tensor(uo[:], psu, bp_sb[:],
                                                op=mybir.AluOpType.add)
                        vo = sp.tile([128, F], dt.bfloat16, tag="vo", name="vo")
                        nc.scalar.copy(vo[:], psv)
                        nc.sync.dma_start(uloc[w * 128:(w + 1) * 128, :], uo[:])
                        nc.sync.dma_start(vloc[:, w, :], vo[:])

            layer(0, None, h1b)
            nc.gpsimd.collective_compute(
                "AllGather", mybir.AluOpType.bypass, replica_groups=rg,
                ins=[h1b[:].opt()], outs=[h1f[:].opt()])
            layer(1, h1f[:], None)
            nc.gpsimd.collective_compute(
                "AllGather", mybir.AluOpType.bypass, replica_groups=rg,
                ins=[uloc[:].opt()], outs=[uf[:].opt()])

            # triplet: out[slot] = u[p(src)] + onehot(lane(dst)) @ v_window
            for w in range(W):
                vw = sp.tile([128, F], dt.bfloat16, tag="vw", name="vw")
                nc.sync.dma_start(vw[:], vloc[:, w, :])
                ut = msgp.tile([128, R, F], dt.bfloat16, tag="ut", name="ut")
                for b in range(R):
                    ch = w * R + b
                    nc.gpsimd.indirect_dma_start(
                        out=ut[:, b, :], out_offset=None, in_=uf[:],
                        in_offset=bass.IndirectOffsetOnAxis(
                            ap=idx_sb[:, ch:ch + 1], axis=0))
                st = stp.tile([128, R, 128], dt.bfloat16, tag="st", name="st")
                nc.sync.dma_start(st[:], slabT_d[:, w * R:(w + 1) * R, :])
                ot = op.tile([128, R, F], dt.bfloat16, tag="ot", name="ot")
                vpst = psaggp.tile([128, 2 * F], dt.float32, tag="vps",
                                   name="vps")
                for b in range(R):
                    vps = vpst[:, (b % 2) * F:(b % 2) * F + F]
                    nc.tensor.matmul(vps, lhsT=st[:, b, :], rhs=vw[:],
                                     start=True, stop=True)
                    nc.vector.tensor_tensor(ot[:, b, :], ut[:, b, :], vps,
                                            op=mybir.AluOpType.add)
                nc.sync.dma_start(tout[:, w * R:(w + 1) * R, :], ot[:])
    nc.compile()
    return nc


def kernel(**inputs):
    from concourse.bass_utils import run_bass_kernel_spmd

    x = np.asarray(inputs["x"], dtype=np.float32)
    ei = np.asarray(inputs["edge_index"], dtype=np.int64)
    et = np.asarray(inputs["edge_type"], dtype=np.int64)
    src, dst = ei[0], ei[1]
    cnt = np.bincount(dst * R + et, minlength=N * R)
    norm = (1.0 / np.maximum(cnt[dst * R + et], 1)).astype(np.float32)

    W, win_of, lane_of, core_of, slots = _plan(src, dst, et, norm)
    C = W * R
    WP = W * 128
    S = C * 128
    nc = _build(W)

    x16 = x.astype(BF16)
    p_of = core_of.astype(np.int64) * WP + win_of.astype(np.int64) * 128 \
        + lane_of.astype(np.int64)

    w1 = np.asarray(inputs["W1"], np.float32).astype(BF16)
    w2 = np.asarray(inputs["W2"], np.float32).astype(BF16)
    r1 = np.asarray(inputs["root1"], np.float32).astype(BF16)
    r2 = np.asarray(inputs["root2"], np.float32).astype(BF16)
    wp = np.asarray(inputs["Wp"], np.float32)
    b1 = np.tile(np.asarray(inputs["b1"], np.float32).reshape(1, F), (128, 1))
    b2 = np.tile(np.asarray(inputs["b2"], np.float32).reshape(1, F), (128, 1))
    bp = np.tile(np.asarray(inputs["bp"], np.float32).reshape(1, F), (128, 1))
    iden = np.eye(128, dtype=BF16)

    in_maps = []
    outmaps = []
    for c in range(NCORES):
        eids, eslot = slots[c]
        es, ed = src[eids], dst[eids]
        ep, ech = eslot & 127, eslot >> 7
        xsh = np.zeros((WP, F), dtype=BF16)
        nodes_c = np.where(core_of == c)[0]
        xsh[win_of[nodes_c] * 128 + lane_of[nodes_c]] = x16[nodes_c]
        xshT = np.ascontiguousarray(
            xsh.reshape(W, 128, 2, 128).transpose(3, 0, 2, 1))
        idx = np.zeros(S, np.int32)
        idx[eslot] = p_of[es]
        msg = np.zeros((128, C, F), dtype=BF16)
        msg[ep, ech] = x16[es]
        slab = np.zeros((128, C, 128), dtype=BF16)
        slab[ep, ech, lane_of[ed]] = norm[eids]
        slabT = np.zeros((128, C, 128), dtype=BF16)
        slabT[lane_of[ed], ech, ep] = 1.0
        outmap = np.full(S, -1, np.int64)
        outmap[eslot] = eids
        outmaps.append(outmap)
        in_maps.append({
            "xshT": xshT, "msg1": msg, "slab": slab, "slabT": slabT,
            "idx": _wrap_idx(idx), "iden": iden,
            "w1": w1, "w2": w2, "r1": r1, "r2": r2,
            "b1": b1, "b2": b2,
            "wpu": wp[:F].astype(BF16), "wpv": wp[F:].astype(BF16), "bp": bp,
        })

    import os
    res = None
    if os.environ.get("BASS_KERNEL_TRACE"):
        try:
            res = run_bass_kernel_spmd(nc, in_maps,
                                       core_ids=list(range(NCORES)), trace=True)
        except Exception:
            res = None
    if res is None:
        res = run_bass_kernel_spmd(nc, in_maps, core_ids=list(range(NCORES)))
    global LAST_EXEC_NS
    LAST_EXEC_NS = res.exec_time_ns
    out = np.zeros((E, F), dtype=np.float32)
    for c in range(NCORES):
        t = np.asarray(res.results[c]["tout"]).astype(np.float32)
        t = t.transpose(1, 0, 2).reshape(S, F)
        om = outmaps[c]
        valid = om >= 0
        out[om[valid]] = t[valid]
    return out


# revision 13
# speedup vs baseline: 1.0023x; 1.0023x over previous
"""RGCN (2-layer, per-(dst,rel) mean aggregation) + triplet projection,
distributed over 8 Trainium2 NeuronCores (one SPMD Bass/Tile program).

Sharding: nodes are assigned to (core, window, lane) by a host-side
best-fit bin packing so that EVERY (window, rel) run has <= 128 edges
(K=1): chunk (w, r) holds exactly the edges with dst in window w and
type r. W windows/core, C = 8*W chunks per layer pass (~96% slot fill).

Aggregate-first: y[w,r] = msg^T @ slab accumulated in PSUM per window
(slab[e, lane(dst)] = 1/cnt), then agg = x @ root + sum_r y_r @ W_r + b,
ReLU. Layer-1 messages are HOST-pregathered; layer-2 gathers h1[p(src)]
per chunk via indirect DMA (int32 idx, one 128-row gather per chunk).

All large DRAM tensors use the device tile layout directly (partition-
major [128, C, ...]) so every stream DMA is a contiguous multi-KB run
per partition — no scatter/transpose patterns on the DMA queues. The
x^T / h1^T tiles needed for the root-term matmuls are produced with
tensor-engine transposes (identity matmul), not transpose-DMAs.

The u/v projections (u = h2 @ Wp[:256] + bp, v = h2 @ Wp[256:]) are
fused into the layer-2 window loop; h2^T stays in SBUF. Only u is
AllGathered. Triplet per chunk: out = u[p(src)] (indirect gather, same
idx tensor as layer 2) + slabT_chunk^T-matmul of the LOCAL window's v
rows (binary one-hot — no gather and no v AllGather). Output in slot
order; host scatters back to edge order.
"""

import numpy as np
import ml_dtypes

BF16 = ml_dtypes.bfloat16

N, R, F, E, NCORES = 50000, 8, 256, 400000, 8
LAST_EXEC_NS = None


def _plan(src, dst, et, norm):
    """Node->core/window/lane packing + edge slot assignment."""
    deg = np.zeros((N, R), dtype=np.int32)
    np.add.at(deg, (dst, et), 1)
    tot = deg.sum(1)

    # 1) node -> core: greedy balance total edges, node cap N/NCORES
    order = np.argsort(-tot, kind="stable")
    core_of = np.full(N, -1, np.int8)
    core_edges = np.zeros(NCORES, np.int64)
    core_nodes_cnt = np.zeros(NCORES, np.int64)
    cap = N // NCORES
    for n in order:
        cands = np.where(core_nodes_cnt < cap)[0]
        c = cands[np.argmin(core_edges[cands])]
        core_of[n] = c
        core_edges[c] += tot[n]
        core_nodes_cnt[c] += 1

    # 2) per-core window packing (best fit decreasing, fixed W, retry W+1)
    def pack(nodes, W):
        dv = deg[nodes]
        o = np.argsort(-dv.max(1).astype(np.int64) * 100000 - dv.sum(1),
                       kind="stable")
        nodes = nodes[o]
        dv = dv[o]
        ws = np.zeros((W, R), np.int32)
        wc = np.zeros(W, np.int32)
        assign = np.zeros(len(nodes), np.int32)
        for i in range(len(nodes)):
            cand = ws + dv[i]
            feas = (cand <= 128).all(1) & (wc < 128)
            if not feas.any():
                return None
            score = (cand.astype(np.int64) ** 2).sum(1)
            score[~feas] = 1 << 60
            w = int(np.argmin(score))
            assign[i] = w
            ws[w] += dv[i]
            wc[w] += 1
        return nodes, assign

    percore_nodes = [np.where(core_of == c)[0] for c in range(NCORES)]
    lb = 0
    for c in range(NCORES):
        nd = percore_nodes[c]
        lb = max(lb, -(-len(nd) // 128), int(-(-deg[nd].sum(0).max() // 128)))
    W = lb
    packed = None
    while packed is None:
        res = [pack(percore_nodes[c], W) for c in range(NCORES)]
        if all(r is not None for r in res):
            packed = res
        else:
            W += 1

    win_of = np.zeros(N, np.int32)
    lane_of = np.zeros(N, np.int32)
    for c in range(NCORES):
        nodes, assign = packed[c]
        o = np.argsort(assign, kind="stable")
        nodes, assign = nodes[o], assign[o]
        lane = np.arange(len(nodes)) - np.searchsorted(assign, assign)
        win_of[nodes] = assign
        lane_of[nodes] = lane

    # 3) edge -> slot (per dst core, chunk (w, r), position in run)
    C = W * R
    ecore = core_of[dst].astype(np.int64)
    ch = win_of[dst].astype(np.int64) * R + et
    gkey = ecore * C + ch
    eorder = np.argsort(gkey, kind="stable")
    gk = gkey[eorder]
    starts = np.zeros(len(gk), np.int64)
    b = np.flatnonzero(np.diff(gk)) + 1
    off = np.arange(len(gk))
    starts[b] = off[b]
    starts = np.maximum.accumulate(starts)
    pos = off - starts
    assert pos.max() < 128, f"run overflow {pos.max()}"
    slot = (gk % C) * 128 + pos

    slots = []
    gcore = gk // C
    for c in range(NCORES):
        m = gcore == c
        slots.append((eorder[m], slot[m]))
    return W, win_of, lane_of, core_of, slots


def _wrap_idx(idx):
    """int32 [C*128] -> [128, C]: chunk c's 128 rows on partitions."""
    return np.ascontiguousarray(idx.reshape(-1, 128).T)


def _build(W):
    import concourse.bass as bass
    import concourse.bacc as bacc
    import concourse.mybir as mybir
    import concourse.tile as tile

    dt = mybir.dt
    C = W * R
    WP = W * 128           # padded nodes per core
    S = C * 128            # slots per core
    nc = bacc.Bacc("TRN2", target_bir_lowering=False, debug=False,
                   num_devices=NCORES)
    AF = mybir.ActivationFunctionType

    xshT = nc.dram_tensor("xshT", [128, W, 2, 128], dt.bfloat16, kind="ExternalInput")
    msg1 = nc.dram_tensor("msg1", [128, C, F], dt.bfloat16, kind="ExternalInput")
    slab_d = nc.dram_tensor("slab", [128, C, 128], dt.bfloat16, kind="ExternalInput")
    slabT_d = nc.dram_tensor("slabT", [128, C, 128], dt.bfloat16, kind="ExternalInput")
    idx_d = nc.dram_tensor("idx", [128, C], dt.int32, kind="ExternalInput")
    iden_d = nc.dram_tensor("iden", [128, 128], dt.bfloat16, kind="ExternalInput")
    w1d = nc.dram_tensor("w1", [R, F, F], dt.bfloat16, kind="ExternalInput")
    w2d = nc.dram_tensor("w2", [R, F, F], dt.bfloat16, kind="ExternalInput")
    r1d = nc.dram_tensor("r1", [F, F], dt.bfloat16, kind="ExternalInput")
    r2d = nc.dram_tensor("r2", [F, F], dt.bfloat16, kind="ExternalInput")
    b1d = nc.dram_tensor("b1", [128, F], dt.float32, kind="ExternalInput")
    b2d = nc.dram_tensor("b2", [128, F], dt.float32, kind="ExternalInput")
    wpud = nc.dram_tensor("wpu", [F, F], dt.bfloat16, kind="ExternalInput")
    wpvd = nc.dram_tensor("wpv", [F, F], dt.bfloat16, kind="ExternalInput")
    bpd = nc.dram_tensor("bp", [128, F], dt.float32, kind="ExternalInput")
    tout = nc.dram_tensor("tout", [128, C, F], dt.bfloat16, kind="ExternalOutput")

    rg = [list(range(NCORES))]

    with tile.TileContext(nc) as tc:
        with (
            tc.tile_pool(name="const", bufs=1) as cp,
            tc.tile_pool(name="msg", bufs=5) as msgp,
            tc.tile_pool(name="stt", bufs=3) as stp,
            tc.tile_pool(name="yw", bufs=2) as yp,
            tc.tile_pool(name="small", bufs=4) as sp,
            tc.tile_pool(name="out", bufs=3) as op,
            tc.tile_pool(name="ps", bufs=1, space="PSUM") as psp,
            tc.tile_pool(name="psagg", bufs=1, space="PSUM") as psaggp,
            tc.tile_pool(name="dram", bufs=1, space="DRAM") as dram,
        ):
            w_sb = [cp.tile([128, 16, F], dt.bfloat16, tag=f"w{i}", name=f"w{i}")
                    for i in range(2)]
            nc.sync.dma_start(w_sb[0][:], w1d.ap().rearrange("r (h p) o -> p (r h) o", p=128))
            nc.sync.dma_start(w_sb[1][:], w2d.ap().rearrange("r (h p) o -> p (r h) o", p=128))
            rt_sb = [cp.tile([128, 2, F], dt.bfloat16, tag=f"rt{i}", name=f"rt{i}")
                     for i in range(2)]
            nc.sync.dma_start(rt_sb[0][:], r1d.ap().rearrange("(h p) o -> p h o", p=128))
            nc.sync.dma_start(rt_sb[1][:], r2d.ap().rearrange("(h p) o -> p h o", p=128))
            b_sb = [cp.tile([128, F], dt.float32, tag=f"b{i}", name=f"b{i}")
                    for i in range(2)]
            nc.sync.dma_start(b_sb[0][:], b1d[:])
            nc.sync.dma_start(b_sb[1][:], b2d[:])
            wpu_sb = cp.tile([128, 2, F], dt.bfloat16, tag="wpu", name="wpu")
            wpv_sb = cp.tile([128, 2, F], dt.bfloat16, tag="wpv", name="wpv")
            nc.sync.dma_start(wpu_sb[:], wpud.ap().rearrange("(h p) o -> p h o", p=128))
            nc.sync.dma_start(wpv_sb[:], wpvd.ap().rearrange("(h p) o -> p h o", p=128))
            bp_sb = cp.tile([128, F], dt.float32, tag="bp", name="bp")
            nc.sync.dma_start(bp_sb[:], bpd[:])
            idx_sb = cp.tile([128, C], dt.int32, tag="idx", name="idx")
            nc.sync.dma_start(idx_sb[:], idx_d[:])
            iden = cp.tile([128, 128], dt.bfloat16, tag="iden", name="iden")
            nc.sync.dma_start(iden[:], iden_d[:])
            # norm slab cache: used by both layer passes
            slab_sb = cp.tile([128, C, 128], dt.bfloat16, tag="slab", name="slab")
            nc.sync.dma_start(slab_sb[:], slab_d[:])

            h1b = dram.tile([WP, F], dt.bfloat16, tag="h1b", name="h1b")
            h1bT = dram.tile([128, W, 2, 128], dt.bfloat16, tag="h1bT", name="h1bT")
            uloc = dram.tile([WP, F], dt.bfloat16, tag="uloc", name="uloc")
            vloc = dram.tile([128, W, F], dt.bfloat16, tag="vloc", name="vloc")
            h1f = dram.tile([NCORES * WP, F], dt.bfloat16, addr_space="Shared",
                            tag="h1f", name="h1f")
            uf = dram.tile([NCORES * WP, F], dt.bfloat16, addr_space="Shared",
                           tag="uf", name="uf")

            def layer(li, table, hout):
                for w in range(W):
                    mt = msgp.tile([128, R, F], dt.bfloat16, tag="mt", name="mt")
                    if li == 0:
                        eng = nc.sync if w % 2 == 0 else nc.scalar
                        eng.dma_start(mt[:], msg1[:, w * R:(w + 1) * R, :])
                    else:
                        for b in range(R):
                            ch = w * R + b
                            nc.gpsimd.indirect_dma_start(
                                out=mt[:, b, :], out_offset=None, in_=table,
                                in_offset=bass.IndirectOffsetOnAxis(
                                    ap=idx_sb[:, ch:ch + 1], axis=0))
                    xt = sp.tile([128, 2, 128], dt.bfloat16, tag="xt", name="xt")
                    if li == 0:
                        nc.sync.dma_start(xt[:], xshT[:, w, :, :])
                    else:
                        nc.sync.dma_start(xt[:], h1bT[:, w, :, :])
                    # rel halves: matmuls of half B overlap PSUM copies of A
                    yq = [[yp.tile([128, 512], dt.bfloat16, tag=f"yq{fh}{hf}",
                                   name=f"yq{fh}{hf}") for hf in range(2)]
                          for fh in range(2)]
                    for half in range(2):
                        ps = [psp.tile([128, 512], dt.float32, tag=f"ps{fh}{half}",
                                       name=f"ps{fh}{half}") for fh in range(2)]
                        for r4 in range(4):
                            r = half * 4 + r4
                            ch = w * R + r
                            for fh in range(2):
                                nc.tensor.matmul(
                                    ps[fh][:, r4 * 128:r4 * 128 + 128],
                                    lhsT=mt[:, r, fh * 128:(fh + 1) * 128],
                                    rhs=slab_sb[:, ch, :],
                                    start=True, stop=True)
                        nc.vector.tensor_copy(yq[0][half][:], ps[0][:])
                        nc.scalar.copy(yq[1][half][:], ps[1][:])
                    # aggregation: roots first (xt ready early, no yq dep)
                    agg = psaggp.tile([128, F], dt.float32, tag="agg", name="agg")
                    for fh in range(2):
                        nc.tensor.matmul(agg[:], lhsT=xt[:, fh, :],
                                         rhs=rt_sb[li][:, fh, :],
                                         start=(fh == 0), stop=False)
                    for half in range(2):
                        for r4 in range(4):
                            r = half * 4 + r4
                            for fh in range(2):
                                nc.tensor.matmul(
                                    agg[:],
                                    lhsT=yq[fh][half][:, r4 * 128:r4 * 128 + 128],
                                    rhs=w_sb[li][:, r * 2 + fh, :],
                                    start=False,
                                    stop=(half == 1 and r4 == 3 and fh == 1))
                    hfb = sp.tile([128, F], dt.bfloat16, tag="hfb", name="hfb")
                    nc.vector.tensor_tensor(hfb[:], agg[:], b_sb[li][:],
                                            op=mybir.AluOpType.add)
                    hw = sp.tile([128, F], dt.bfloat16, tag="hw", name="hw")
                    nc.scalar.activation(hw[:], hfb[:], AF.Relu)
                    # h^T via tensor-engine transpose of the pre-relu sum;
                    # relu commutes with transpose and is re-applied below
                    pst = psaggp.tile([128, F], dt.bfloat16, tag="pst", name="pst")
                    for fh in range(2):
                        nc.tensor.transpose(pst[:, fh * 128:(fh + 1) * 128],
                                            hfb[:, fh * 128:(fh + 1) * 128],
                                            iden[:])
                    hT = sp.tile([128, 2, 128], dt.bfloat16, tag="hT", name="hT")
                    nc.vector.tensor_scalar_max(
                        hT[:].rearrange("p a b -> p (a b)"), pst[:], 0.0)
                    if li == 0:
                        nc.sync.dma_start(hout[w * 128:(w + 1) * 128, :], hw[:])
                        nc.sync.dma_start(h1bT[:, w, :, :], hT[:])
                    else:
                        # fused u/v projections for this window (h2^T in SBUF)
                        psuv = psaggp.tile([128, 2 * F], dt.float32, tag="psuv",
                                           name="psuv")
                        psu = psuv[:, 0:F]
                        psv = psuv[:, F:2 * F]
                        # chains must not interleave within one PSUM bank
                        for fh in range(2):
                            nc.tensor.matmul(psu, lhsT=hT[:, fh, :],
                                             rhs=wpu_sb[:, fh, :],
                                             start=(fh == 0), stop=(fh == 1))
                        for fh in range(2):
                            nc.tensor.matmul(psv, lhsT=hT[:, fh, :],
                                             rhs=wpv_sb[:, fh, :],
                                             start=(fh == 0), stop=(fh == 1))
                        uo = sp.tile([128, F], dt.bfloat16, tag="uo", name="uo")
                        nc.vector.tensor_tensor(uo[:], psu, bp_sb[:],
                                                op=mybir.AluOpType.add)
                        vo = sp.tile([128, F], dt.bfloat16, tag="vo", name="vo")
                        nc.scalar.copy(vo[:], psv)
                        nc.sync.dma_start(uloc[w * 128:(w + 1) * 128, :], uo[:])
                        nc.sync.dma_start(vloc[:, w, :], vo[:])

            layer(0, None, h1b)
            nc.gpsimd.collective_compute(
                "AllGather", mybir.AluOpType.bypass, replica_groups=rg,
                ins=[h1b[:].opt()], outs=[h1f[:].opt()])
            layer(1, h1f[:], None)
            nc.gpsimd.collective_compute(
                "AllGather", mybir.AluOpType.bypass, replica_groups=rg,
                ins=[uloc[:].opt()], outs=[uf[:].opt()])

            # triplet: out[slot] = u[p(src)] + onehot(lane(dst)) @ v_window
            for w in range(W):
                vw = sp.tile([128, F], dt.bfloat16, tag="vw", name="vw")
                nc.sync.dma_start(vw[:], vloc[:, w, :])
                ut = msgp.tile([128, R, F], dt.bfloat16, tag="ut", name="ut")
                for b in range(R):
                    ch = w * R + b
                    nc.gpsimd.indirect_dma_start(
                        out=ut[:, b, :], out_offset=None, in_=uf[:],
                        in_offset=bass.IndirectOffsetOnAxis(
                            ap=idx_sb[:, ch:ch + 1], axis=0))
                st = stp.tile([128, R, 128], dt.bfloat16, tag="st", name="st")
                nc.sync.dma_start(st[:], slabT_d[:, w * R:(w + 1) * R, :])
                ot = op.tile([128, R, F], dt.bfloat16, tag="ot", name="ot")
                vpst = psaggp.tile([128, 2 * F], dt.float32, tag="vps",
                                   name="vps")
                for b in range(R):
                    vps = vpst[:, (b % 2) * F:(b % 2) * F + F]
                    nc.tensor.matmul(vps, lhsT=st[:, b, :], rhs=vw[:],
                                     start=True, stop=True)
                    nc.vector.tensor_tensor(ot[:, b, :], ut[:, b, :], vps,
                                            op=mybir.AluOpType.add)
                nc.sync.dma_start(tout[:, w * R:(w + 1) * R, :], ot[:])
    nc.compile()
    return nc


def kernel(**inputs):
    from concourse.bass_utils import run_bass_kernel_spmd

    x = np.asarray(inputs["x"], dtype=np.float32)
    ei = np.asarray(inputs["edge_index"], dtype=np.int64)
    et = np.asarray(inputs["edge_type"], dtype=np.int64)
    src, dst = ei[0], ei[1]
    cnt = np.bincount(dst * R + et, minlength=N * R)
    norm = (1.0 / np.maximum(cnt[dst * R + et], 1)).astype(np.float32)

    W, win_of, lane_of, core_of, slots = _plan(src, dst, et, norm)
    C = W * R
    WP = W * 128
    S = C * 128
    nc = _build(W)

    x16 = x.astype(BF16)
    p_of = core_of.astype(np.int64) * WP + win_of.astype(np.int64) * 128 \
        + lane_of.astype(np.int64)

    w1 = np.asarray(inputs["W1"], np.float32).astype(BF16)
    w2 = np.asarray(inputs["W2"], np.float32).astype(BF16)
    r1 = np.asarray(inputs["root1"], np.float32).astype(BF16)
    r2 = np.asarray(inputs["root2"], np.float32).astype(BF16)
    wp = np.asarray(inputs["Wp"], np.float32)
    b1 = np.tile(np.asarray(inputs["b1"], np.float32).reshape(1, F), (128, 1))
    b2 = np.tile(np.asarray(inputs["b2"], np.float32).reshape(1, F), (128, 1))
    bp = np.tile(np.asarray(inputs["bp"], np.float32).reshape(1, F), (128, 1))
    iden = np.eye(128, dtype=BF16)

    in_maps = []
    outmaps = []
    for c in range(NCORES):
        eids, eslot = slots[c]
        es, ed = src[eids], dst[eids]
        ep, ech = eslot & 127, eslot >> 7
        xsh = np.zeros((WP, F), dtype=BF16)
        nodes_c = np.where(core_of == c)[0]
        xsh[win_of[nodes_c] * 128 + lane_of[nodes_c]] = x16[nodes_c]
        xshT = np.ascontiguousarray(
            xsh.reshape(W, 128, 2, 128).transpose(3, 0, 2, 1))
        idx = np.zeros(S, np.int32)
        idx[eslot] = p_of[es]
        msg = np.zeros((128, C, F), dtype=BF16)
        msg[ep, ech] = x16[es]
        slab = np.zeros((128, C, 128), dtype=BF16)
        slab[ep, ech, lane_of[ed]] = norm[eids]
        slabT = np.zeros((128, C, 128), dtype=BF16)
        slabT[lane_of[ed], ech, ep] = 1.0
        outmap = np.full(S, -1, np.int64)
        outmap[eslot] = eids
        outmaps.append(outmap)
        in_maps.append({
            "xshT": xshT, "msg1": msg, "slab": slab, "slabT": slabT,
            "idx": _wrap_idx(idx), "iden": iden,
            "w1": w1, "w2": w2, "r1": r1, "r2": r2,
            "b1": b1, "b2": b2,
            "wpu": wp[:F].astype(BF16), "wpv": wp[F:].astype(BF16), "bp": bp,
        })

    import os
    res = None
    if os.environ.get("BASS_KERNEL_TRACE"):
        try:
            res = run_bass_kernel_spmd(nc, in_maps,
                                       core_ids=list(range(NCORES)), trace=True)
        except Exception:
            res = None
    if res is None:
        res = run_bass_kernel_spmd(nc, in_maps, core_ids=list(range(NCORES)))
    global LAST_EXEC_NS
    LAST_EXEC_NS = res.exec_time_ns
    out = np.zeros((E, F), dtype=np.float32)
    for c in range(NCORES):
        t = np.asarray(res.results[c]["tout"]).astype(np.float32)
        t = t.transpose(1, 0, 2).reshape(S, F)
        om = outmaps[c]
        valid = om >= 0
        out[om[valid]] = t[valid]
    return out


# revision 14
# speedup vs baseline: 1.0052x; 1.0029x over previous
"""RGCN (2-layer, per-(dst,rel) mean aggregation) + triplet projection,
distributed over 8 Trainium2 NeuronCores (one SPMD Bass/Tile program).

Sharding: nodes are assigned to (core, window, lane) by a host-side
best-fit bin packing so that EVERY (window, rel) run has <= 128 edges
(K=1): chunk (w, r) holds exactly the edges with dst in window w and
type r. W windows/core, C = 8*W chunks per layer pass (~96% slot fill).

Aggregate-first: y[w,r] = msg^T @ slab accumulated in PSUM per window
(slab[e, lane(dst)] = 1/cnt), then agg = x @ root + sum_r y_r @ W_r + b,
ReLU. Layer-1 messages are HOST-pregathered; layer-2 gathers h1[p(src)]
per chunk via indirect DMA (int32 idx, one 128-row gather per chunk).

All large DRAM tensors use the device tile layout directly (partition-
major [128, C, ...]) so every stream DMA is a contiguous multi-KB run
per partition — no scatter/transpose patterns on the DMA queues. The
x^T / h1^T tiles needed for the root-term matmuls are produced with
tensor-engine transposes (identity matmul), not transpose-DMAs.

The u/v projections (u = h2 @ Wp[:256] + bp, v = h2 @ Wp[256:]) are
fused into the layer-2 window loop; h2^T stays in SBUF. Only u is
AllGathered. Triplet per chunk: out = u[p(src)] (indirect gather, same
idx tensor as layer 2) + slabT_chunk^T-matmul of the LOCAL window's v
rows (binary one-hot — no gather and no v AllGather). Output in slot
order; host scatters back to edge order.
"""

import numpy as np
import ml_dtypes

BF16 = ml_dtypes.bfloat16

N, R, F, E, NCORES = 50000, 8, 256, 400000, 8
LAST_EXEC_NS = None


def _plan(src, dst, et, norm):
    """Node->core/window/lane packing + edge slot assignment."""
    deg = np.zeros((N, R), dtype=np.int32)
    np.add.at(deg, (dst, et), 1)
    tot = deg.sum(1)

    # 1) node -> core: greedy balance total edges, node cap N/NCORES
    order = np.argsort(-tot, kind="stable")
    core_of = np.full(N, -1, np.int8)
    core_edges = np.zeros(NCORES, np.int64)
    core_nodes_cnt = np.zeros(NCORES, np.int64)
    cap = N // NCORES
    for n in order:
        cands = np.where(core_nodes_cnt < cap)[0]
        c = cands[np.argmin(core_edges[cands])]
        core_of[n] = c
        core_edges[c] += tot[n]
        core_nodes_cnt[c] += 1

    # 2) per-core window packing (best fit decreasing, fixed W, retry W+1)
    def pack(nodes, W):
        dv = deg[nodes]
        o = np.argsort(-dv.max(1).astype(np.int64) * 100000 - dv.sum(1),
                       kind="stable")
        nodes = nodes[o]
        dv = dv[o]
        ws = np.zeros((W, R), np.int32)
        wc = np.zeros(W, np.int32)
        assign = np.zeros(len(nodes), np.int32)
        for i in range(len(nodes)):
            cand = ws + dv[i]
            feas = (cand <= 128).all(1) & (wc < 128)
            if not feas.any():
                return None
            score = (cand.astype(np.int64) ** 2).sum(1)
            score[~feas] = 1 << 60
            w = int(np.argmin(score))
            assign[i] = w
            ws[w] += dv[i]
            wc[w] += 1
        return nodes, assign

    percore_nodes = [np.where(core_of == c)[0] for c in range(NCORES)]
    lb = 0
    for c in range(NCORES):
        nd = percore_nodes[c]
        lb = max(lb, -(-len(nd) // 128), int(-(-deg[nd].sum(0).max() // 128)))
    W = lb
    packed = None
    while packed is None:
        res = [pack(percore_nodes[c], W) for c in range(NCORES)]
        if all(r is not None for r in res):
            packed = res
        else:
            W += 1

    win_of = np.zeros(N, np.int32)
    lane_of = np.zeros(N, np.int32)
    for c in range(NCORES):
        nodes, assign = packed[c]
        o = np.argsort(assign, kind="stable")
        nodes, assign = nodes[o], assign[o]
        lane = np.arange(len(nodes)) - np.searchsorted(assign, assign)
        win_of[nodes] = assign
        lane_of[nodes] = lane

    # 3) edge -> slot (per dst core, chunk (w, r), position in run)
    C = W * R
    ecore = core_of[dst].astype(np.int64)
    ch = win_of[dst].astype(np.int64) * R + et
    gkey = ecore * C + ch
    eorder = np.argsort(gkey, kind="stable")
    gk = gkey[eorder]
    starts = np.zeros(len(gk), np.int64)
    b = np.flatnonzero(np.diff(gk)) + 1
    off = np.arange(len(gk))
    starts[b] = off[b]
    starts = np.maximum.accumulate(starts)
    pos = off - starts
    assert pos.max() < 128, f"run overflow {pos.max()}"
    slot = (gk % C) * 128 + pos

    slots = []
    gcore = gk // C
    for c in range(NCORES):
        m = gcore == c
        slots.append((eorder[m], slot[m]))
    return W, win_of, lane_of, core_of, slots


def _wrap_idx(idx):
    """int32 [C*128] -> [128, C]: chunk c's 128 rows on partitions."""
    return np.ascontiguousarray(idx.reshape(-1, 128).T)


def _build(W):
    import concourse.bass as bass
    import concourse.bacc as bacc
    import concourse.mybir as mybir
    import concourse.tile as tile

    dt = mybir.dt
    C = W * R
    WP = W * 128           # padded nodes per core
    S = C * 128            # slots per core
    nc = bacc.Bacc("TRN2", target_bir_lowering=False, debug=False,
                   num_devices=NCORES)
    AF = mybir.ActivationFunctionType

    xshT = nc.dram_tensor("xshT", [128, W, 2, 128], dt.bfloat16, kind="ExternalInput")
    msg1 = nc.dram_tensor("msg1", [128, C, F], dt.bfloat16, kind="ExternalInput")
    slab_d = nc.dram_tensor("slab", [128, C, 128], dt.bfloat16, kind="ExternalInput")
    slabT_d = nc.dram_tensor("slabT", [128, C, 128], dt.bfloat16, kind="ExternalInput")
    idx_d = nc.dram_tensor("idx", [128, C], dt.int32, kind="ExternalInput")
    iden_d = nc.dram_tensor("iden", [128, 128], dt.bfloat16, kind="ExternalInput")
    w1d = nc.dram_tensor("w1", [R, F, F], dt.bfloat16, kind="ExternalInput")
    w2d = nc.dram_tensor("w2", [R, F, F], dt.bfloat16, kind="ExternalInput")
    r1d = nc.dram_tensor("r1", [F, F], dt.bfloat16, kind="ExternalInput")
    r2d = nc.dram_tensor("r2", [F, F], dt.bfloat16, kind="ExternalInput")
    b1d = nc.dram_tensor("b1", [128, F], dt.float32, kind="ExternalInput")
    b2d = nc.dram_tensor("b2", [128, F], dt.float32, kind="ExternalInput")
    wpud = nc.dram_tensor("wpu", [F, F], dt.bfloat16, kind="ExternalInput")
    wpvd = nc.dram_tensor("wpv", [F, F], dt.bfloat16, kind="ExternalInput")
    bpd = nc.dram_tensor("bp", [128, F], dt.float32, kind="ExternalInput")
    tout = nc.dram_tensor("tout", [128, C, F], dt.bfloat16, kind="ExternalOutput")

    rg = [list(range(NCORES))]

    with tile.TileContext(nc) as tc:
        with (
            tc.tile_pool(name="const", bufs=1) as cp,
            tc.tile_pool(name="msg", bufs=5) as msgp,
            tc.tile_pool(name="stt", bufs=3) as stp,
            tc.tile_pool(name="yw", bufs=2) as yp,
            tc.tile_pool(name="small", bufs=4) as sp,
            tc.tile_pool(name="out", bufs=3) as op,
            tc.tile_pool(name="ps", bufs=1, space="PSUM") as psp,
            tc.tile_pool(name="psagg", bufs=1, space="PSUM") as psaggp,
            tc.tile_pool(name="dram", bufs=1, space="DRAM") as dram,
        ):
            w_sb = [cp.tile([128, 16, F], dt.bfloat16, tag=f"w{i}", name=f"w{i}")
                    for i in range(2)]
            nc.sync.dma_start(w_sb[0][:], w1d.ap().rearrange("r (h p) o -> p (r h) o", p=128))
            nc.sync.dma_start(w_sb[1][:], w2d.ap().rearrange("r (h p) o -> p (r h) o", p=128))
            rt_sb = [cp.tile([128, 2, F], dt.bfloat16, tag=f"rt{i}", name=f"rt{i}")
                     for i in range(2)]
            nc.sync.dma_start(rt_sb[0][:], r1d.ap().rearrange("(h p) o -> p h o", p=128))
            nc.sync.dma_start(rt_sb[1][:], r2d.ap().rearrange("(h p) o -> p h o", p=128))
            b_sb = [cp.tile([128, F], dt.float32, tag=f"b{i}", name=f"b{i}")
                    for i in range(2)]
            nc.sync.dma_start(b_sb[0][:], b1d[:])
            nc.sync.dma_start(b_sb[1][:], b2d[:])
            wpu_sb = cp.tile([128, 2, F], dt.bfloat16, tag="wpu", name="wpu")
            wpv_sb = cp.tile([128, 2, F], dt.bfloat16, tag="wpv", name="wpv")
            nc.sync.dma_start(wpu_sb[:], wpud.ap().rearrange("(h p) o -> p h o", p=128))
            nc.sync.dma_start(wpv_sb[:], wpvd.ap().rearrange("(h p) o -> p h o", p=128))
            bp_sb = cp.tile([128, F], dt.float32, tag="bp", name="bp")
            nc.sync.dma_start(bp_sb[:], bpd[:])
            idx_sb = cp.tile([128, C], dt.int32, tag="idx", name="idx")
            nc.sync.dma_start(idx_sb[:], idx_d[:])
            iden = cp.tile([128, 128], dt.bfloat16, tag="iden", name="iden")
            nc.sync.dma_start(iden[:], iden_d[:])
            # norm slab cache: used by both layer passes; loaded in two
            # halves so window 0 only waits for the first half
            CH = C // 2
            slab_sbs = [cp.tile([128, CH, 128], dt.bfloat16, tag=f"slab{i}",
                                name=f"slab{i}") for i in range(2)]
            nc.sync.dma_start(slab_sbs[0][:], slab_d[:, 0:CH, :])
            nc.sync.dma_start(slab_sbs[1][:], slab_d[:, CH:C, :])

            def slab_ap(ch):
                return slab_sbs[ch // CH][:, ch % CH, :]

            h1b = dram.tile([WP, F], dt.bfloat16, tag="h1b", name="h1b")
            h1bT = dram.tile([128, W, 2, 128], dt.bfloat16, tag="h1bT", name="h1bT")
            uloc = dram.tile([WP, F], dt.bfloat16, tag="uloc", name="uloc")
            vloc = dram.tile([128, W, F], dt.bfloat16, tag="vloc", name="vloc")
            h1f = dram.tile([NCORES * WP, F], dt.bfloat16, addr_space="Shared",
                            tag="h1f", name="h1f")
            uf = dram.tile([NCORES * WP, F], dt.bfloat16, addr_space="Shared",
                           tag="uf", name="uf")

            def layer(li, table, hout):
                for w in range(W):
                    mth = [msgp.tile([128, 4, F], dt.bfloat16, tag=f"mt{i}",
                                     name=f"mt{i}") for i in range(2)]
                    if li == 0:
                        eng = nc.sync if w % 2 == 0 else nc.scalar
                        for i in range(2):
                            eng.dma_start(
                                mth[i][:],
                                msg1[:, w * R + 4 * i:w * R + 4 * i + 4, :])
                    else:
                        for b in range(R):
                            ch = w * R + b
                            nc.gpsimd.indirect_dma_start(
                                out=mth[b // 4][:, b % 4, :], out_offset=None,
                                in_=table,
                                in_offset=bass.IndirectOffsetOnAxis(
                                    ap=idx_sb[:, ch:ch + 1], axis=0))
                    xt = sp.tile([128, 2, 128], dt.bfloat16, tag="xt", name="xt")
                    if li == 0:
                        nc.sync.dma_start(xt[:], xshT[:, w, :, :])
                    else:
                        nc.sync.dma_start(xt[:], h1bT[:, w, :, :])
                    # rel halves: matmuls of half B overlap PSUM copies of A
                    yq = [[yp.tile([128, 512], dt.bfloat16, tag=f"yq{fh}{hf}",
                                   name=f"yq{fh}{hf}") for hf in range(2)]
                          for fh in range(2)]
                    for half in range(2):
                        ps = [psp.tile([128, 512], dt.float32, tag=f"ps{fh}{half}",
                                       name=f"ps{fh}{half}") for fh in range(2)]
                        for r4 in range(4):
                            r = half * 4 + r4
                            ch = w * R + r
                            for fh in range(2):
                                nc.tensor.matmul(
                                    ps[fh][:, r4 * 128:r4 * 128 + 128],
                                    lhsT=mth[r // 4][:, r % 4, fh * 128:(fh + 1) * 128],
                                    rhs=slab_ap(ch),
                                    start=True, stop=True)
                        nc.vector.tensor_copy(yq[0][half][:], ps[0][:])
                        nc.scalar.copy(yq[1][half][:], ps[1][:])
                    # aggregation: roots first (xt ready early, no yq dep)
                    agg = psaggp.tile([128, F], dt.float32, tag="agg", name="agg")
                    for fh in range(2):
                        nc.tensor.matmul(agg[:], lhsT=xt[:, fh, :],
                                         rhs=rt_sb[li][:, fh, :],
                                         start=(fh == 0), stop=False)
                    for half in range(2):
                        for r4 in range(4):
                            r = half * 4 + r4
                            for fh in range(2):
                                nc.tensor.matmul(
                                    agg[:],
                                    lhsT=yq[fh][half][:, r4 * 128:r4 * 128 + 128],
                                    rhs=w_sb[li][:, r * 2 + fh, :],
                                    start=False,
                                    stop=(half == 1 and r4 == 3 and fh == 1))
                    hfb = sp.tile([128, F], dt.bfloat16, tag="hfb", name="hfb")
                    nc.vector.tensor_tensor(hfb[:], agg[:], b_sb[li][:],
                                            op=mybir.AluOpType.add)
                    hw = sp.tile([128, F], dt.bfloat16, tag="hw", name="hw")
                    nc.scalar.activation(hw[:], hfb[:], AF.Relu)
                    # h^T via tensor-engine transpose of the pre-relu sum;
                    # relu commutes with transpose and is re-applied below
                    pst = psaggp.tile([128, F], dt.bfloat16, tag="pst", name="pst")
                    for fh in range(2):
                        nc.tensor.transpose(pst[:, fh * 128:(fh + 1) * 128],
                                            hfb[:, fh * 128:(fh + 1) * 128],
                                            iden[:])
                    hT = sp.tile([128, 2, 128], dt.bfloat16, tag="hT", name="hT")
                    nc.vector.tensor_scalar_max(
                        hT[:].rearrange("p a b -> p (a b)"), pst[:], 0.0)
                    if li == 0:
                        nc.sync.dma_start(hout[w * 128:(w + 1) * 128, :], hw[:])
                        nc.sync.dma_start(h1bT[:, w, :, :], hT[:])
                    else:
                        # fused u/v projections for this window (h2^T in SBUF)
                        psuv = psaggp.tile([128, 2 * F], dt.float32, tag="psuv",
                                           name="psuv")
                        psu = psuv[:, 0:F]
                        psv = psuv[:, F:2 * F]
                        # chains must not interleave within one PSUM bank
                        for fh in range(2):
                            nc.tensor.matmul(psu, lhsT=hT[:, fh, :],
                                             rhs=wpu_sb[:, fh, :],
                                             start=(fh == 0), stop=(fh == 1))
                        for fh in range(2):
                            nc.tensor.matmul(psv, lhsT=hT[:, fh, :],
                                             rhs=wpv_sb[:, fh, :],
                                             start=(fh == 0), stop=(fh == 1))
                        uo = sp.tile([128, F], dt.bfloat16, tag="uo", name="uo")
                        nc.vector.tensor_tensor(uo[:], psu, bp_sb[:],
                                                op=mybir.AluOpType.add)
                        vo = sp.tile([128, F], dt.bfloat16, tag="vo", name="vo")
                        nc.scalar.copy(vo[:], psv)
                        nc.sync.dma_start(uloc[w * 128:(w + 1) * 128, :], uo[:])
                        nc.sync.dma_start(vloc[:, w, :], vo[:])

            layer(0, None, h1b)
            nc.gpsimd.collective_compute(
                "AllGather", mybir.AluOpType.bypass, replica_groups=rg,
                ins=[h1b[:].opt()], outs=[h1f[:].opt()])
            layer(1, h1f[:], None)
            nc.gpsimd.collective_compute(
                "AllGather", mybir.AluOpType.bypass, replica_groups=rg,
                ins=[uloc[:].opt()], outs=[uf[:].opt()])

            # triplet: out[slot] = u[p(src)] + onehot(lane(dst)) @ v_window
            for w in range(W):
                vw = sp.tile([128, F], dt.bfloat16, tag="vw", name="vw")
                nc.sync.dma_start(vw[:], vloc[:, w, :])
                uth = [msgp.tile([128, 4, F], dt.bfloat16, tag=f"ut{i}",
                                 name=f"ut{i}") for i in range(2)]
                for b in range(R):
                    ch = w * R + b
                    nc.gpsimd.indirect_dma_start(
                        out=uth[b // 4][:, b % 4, :], out_offset=None, in_=uf[:],
                        in_offset=bass.IndirectOffsetOnAxis(
                            ap=idx_sb[:, ch:ch + 1], axis=0))
                st = stp.tile([128, R, 128], dt.bfloat16, tag="st", name="st")
                nc.sync.dma_start(st[:], slabT_d[:, w * R:(w + 1) * R, :])
                ot = op.tile([128, R, F], dt.bfloat16, tag="ot", name="ot")
                vpst = psaggp.tile([128, 2 * F], dt.float32, tag="vps",
                                   name="vps")
                for b in range(R):
                    vps = vpst[:, (b % 2) * F:(b % 2) * F + F]
                    nc.tensor.matmul(vps, lhsT=st[:, b, :], rhs=vw[:],
                                     start=True, stop=True)
                    nc.vector.tensor_tensor(ot[:, b, :], uth[b // 4][:, b % 4, :], vps,
                                            op=mybir.AluOpType.add)
                nc.sync.dma_start(tout[:, w * R:(w + 1) * R, :], ot[:])
    nc.compile()
    return nc


def kernel(**inputs):
    from concourse.bass_utils import run_bass_kernel_spmd

    x = np.asarray(inputs["x"], dtype=np.float32)
    ei = np.asarray(inputs["edge_index"], dtype=np.int64)
    et = np.asarray(inputs["edge_type"], dtype=np.int64)
    src, dst = ei[0], ei[1]
    cnt = np.bincount(dst * R + et, minlength=N * R)
    norm = (1.0 / np.maximum(cnt[dst * R + et], 1)).astype(np.float32)

    W, win_of, lane_of, core_of, slots = _plan(src, dst, et, norm)
    C = W * R
    WP = W * 128
    S = C * 128
    nc = _build(W)

    x16 = x.astype(BF16)
    p_of = core_of.astype(np.int64) * WP + win_of.astype(np.int64) * 128 \
        + lane_of.astype(np.int64)

    w1 = np.asarray(inputs["W1"], np.float32).astype(BF16)
    w2 = np.asarray(inputs["W2"], np.float32).astype(BF16)
    r1 = np.asarray(inputs["root1"], np.float32).astype(BF16)
    r2 = np.asarray(inputs["root2"], np.float32).astype(BF16)
    wp = np.asarray(inputs["Wp"], np.float32)
    b1 = np.tile(np.asarray(inputs["b1"], np.float32).reshape(1, F), (128, 1))
    b2 = np.tile(np.asarray(inputs["b2"], np.float32).reshape(1, F), (128, 1))
    bp = np.tile(np.asarray(inputs["bp"], np.float32).reshape(1, F), (128, 1))
    iden = np.eye(128, dtype=BF16)

    in_maps = []
    outmaps = []
    for c in range(NCORES):
        eids, eslot = slots[c]
        es, ed = src[eids], dst[eids]
        ep, ech = eslot & 127, eslot >> 7
        xsh = np.zeros((WP, F), dtype=BF16)
        nodes_c = np.where(core_of == c)[0]
        xsh[win_of[nodes_c] * 128 + lane_of[nodes_c]] = x16[nodes_c]
        xshT = np.ascontiguousarray(
            xsh.reshape(W, 128, 2, 128).transpose(3, 0, 2, 1))
        idx = np.zeros(S, np.int32)
        idx[eslot] = p_of[es]
        msg = np.zeros((128, C, F), dtype=BF16)
        msg[ep, ech] = x16[es]
        slab = np.zeros((128, C, 128), dtype=BF16)
        slab[ep, ech, lane_of[ed]] = norm[eids]
        slabT = np.zeros((128, C, 128), dtype=BF16)
        slabT[lane_of[ed], ech, ep] = 1.0
        outmap = np.full(S, -1, np.int64)
        outmap[eslot] = eids
        outmaps.append(outmap)
        in_maps.append({
            "xshT": xshT, "msg1": msg, "slab": slab, "slabT": slabT,
            "idx": _wrap_idx(idx), "iden": iden,
            "w1": w1, "w2": w2, "r1": r1, "r2": r2,
            "b1": b1, "b2": b2,
            "wpu": wp[:F].astype(BF16), "wpv": wp[F:].astype(BF16), "bp": bp,
        })

    import os
    res = None
    if os.environ.get("BASS_KERNEL_TRACE"):
        try:
            res = run_bass_kernel_spmd(nc, in_maps,
                                       core_ids=list(range(NCORES)), trace=True)
        except Exception:
            res = None
    if res is None:
        res = run_bass_kernel_spmd(nc, in_maps, core_ids=list(range(NCORES)))
    global LAST_EXEC_NS
    LAST_EXEC_NS = res.exec_time_ns
    out = np.zeros((E, F), dtype=np.float32)
    for c in range(NCORES):
        t = np.asarray(res.results[c]["tout"]).astype(np.float32)
        t = t.transpose(1, 0, 2).reshape(S, F)
        om = outmaps[c]
        valid = om >= 0
        out[om[valid]] = t[valid]
    return out
